# revision 9
# baseline (speedup 1.0000x reference)
"""DeepSeek-V2 normal MoE layer on 8 Trainium2 NeuronCores.

Expert-parallel sharding: core e holds expert e's weights (cast to bf16).
The router (tiny [T,E] matmul + softmax + top-k) runs on the host in fp32 —
it decides which token rows are copied to which core. Each core receives its
routed tokens (bf16, host-packed so every DMA moves contiguous partition
rows). On device, each core computes the gated-SiLU expert MLP for its
tokens (three 2048/1408-contraction matmul phases in bf16 with fp32 PSUM
accumulation, feature-major layout so no on-device transposes are needed)
and returns raw bf16 per-expert outputs; the host applies the renormalized
top-k combine weights during its fp32 scatter-add.

DMA plan. Measured on HW: each logical DMA queue sustains a FLAT rate
regardless of instruction size (~120-131 GB/s for the GpSimd SWDGE queue,
~180 GB/s for the Sync/Scalar HWDGE queues), and exactly three queues can
serve plain loads. The tensor engine needs weights at 134 GB/s + the 2.3MB
token block up front, so no single queue can feed it: the streams are
balanced across all three queues with the front packed by deadline:
 - Sync:   oct0, wgA(it0), oct2, oct4, oct6, then all of w_down (prefetched
   during phase 1 so the phase boundary never waits on DMA).
 - Scalar: wuA(it0), oct1, oct3, wuB(it0), oct5, oct7, then the B weight
   halves [wgB|wuB] for it>=1 (one 512KB instruction each, 67 GB/s of its
   ~180), then y-tile stores in phase 2.
 - GpSimd: wgB(it0), then the A halves [wgA|wuA] for it>=1 (67 GB/s of its
   ~125).
First matmul data lands ~11.5us; every stream has delivery margin instead
of rate-matching the tensor engine. A- and B-halves live in separate tile
rings so their dependency cones stay independent.

Per-core capacity C = max tokens routed to any expert (rounded up to 2);
pad token columns are zero so their output is zero and the host ignores
them. A ~2.5us PE warmup bridges preamble-end to first-data so the HAM
clock gate boosts 1.2->2.4 GHz once, early.
"""

import numpy as np
import ml_dtypes


def _ensure_ntff_hook():
    """This image's antenv package lacks axon_hooks, but concourse's
    run_bass_kernel_spmd unconditionally imports it when BASS_TRACE is set.
    Provide the module (and the ctypes NTFF hook from trn_agent_boot, when
    available) so tracing works instead of crashing. Idempotent; never
    overwrites an existing module."""
    import sys
    import types
    try:
        import antenv  # noqa: F401
    except ImportError:
        return
    if "antenv.axon_hooks" in sys.modules:
        return
    try:
        import antenv.axon_hooks  # noqa: F401
        return
    except ImportError:
        pass
    mod = types.ModuleType("antenv.axon_hooks")
    holder = {"h": None}
    mod.set_axon_ntff_profile_hook = lambda h: holder.__setitem__("h", h)
    mod.get_axon_ntff_profile_hook = lambda: holder.get("h")
    sys.modules["antenv.axon_hooks"] = mod
    import antenv as _a
    _a.axon_hooks = mod
    try:
        from trn_agent_boot.trn_boot import _ntff_profile_via_ctypes
        hook = _ntff_profile_via_ctypes("/opt/axon/libaxon_pjrt.so")
        if hook is not None:
            mod.set_axon_ntff_profile_hook(hook)
    except Exception:
        pass


_ensure_ntff_hook()

H = 2048
I_DIM = 1408
E = 8
P = 128
HT = H // P      # 16
IT = I_DIM // P  # 11
HH = HT // 2     # 8

_compiled = {}
last_results = None


def _chunks(C):
    """Token-column chunks of <=512 (one PSUM bank / max moving free dim).
    Near-equal sizes with every chunk >=128 where possible: a matmul
    narrower than ~60 cycles pays the NX-dispatch floor (~25ns at 2.4GHz
    regardless of width), so (426,128) beats (512,42)."""
    n = (C + 511) // 512
    if n == 1:
        return [(0, C)]
    out = []
    s = 0
    for i in range(n):
        w = C // n + (1 if i < C % n else 0)
        if i == n - 1:
            w = C - s
        out.append((s, w))
        s += w
    return out


def _build(C):
    import concourse.bacc as bacc
    import concourse.mybir as mybir
    import concourse.tile as tile

    dt = mybir.dt
    nc = bacc.Bacc("TRN2", target_bir_lowering=False)
    # Layouts (host pre-tiled so every DMA moves contiguous partition rows):
    #   xg  [P, HT*C]      tokens, h-tile h at cols h*C..(h+1)*C
    #   wgu [IT, P, 32*P]  per-it [wgA(h0-7) | wuA(h0-7) | wgB(h8-15) | wuB]
    #   wdp [HT//2, P, 2*IT*P]  two ht-tiles of w_down per block
    #   yt  [H, C] bf16    raw expert output (combine weight applied on host)
    xg = nc.dram_tensor("xg", [P, HT * C], dt.bfloat16, kind="ExternalInput")
    wgu = nc.dram_tensor("wgu", [IT, P, 2 * HT * P], dt.bfloat16,
                         kind="ExternalInput")
    wdp = nc.dram_tensor("wdp", [HT // 2, P, 2 * IT * P], dt.bfloat16,
                         kind="ExternalInput")
    yt = nc.dram_tensor("yt", [H, C], dt.bfloat16, kind="ExternalOutput")

    ch = _chunks(C)

    HB = HH * P  # 1024 cols per [wg|wu] half-block

    with tile.TileContext(nc) as tc:
        with (
            tc.tile_pool(name="xpool", bufs=1) as xpool,
            tc.tile_pool(name="apool", bufs=1) as apool,
            tc.tile_pool(name="wapool", bufs=3) as wapool,
            tc.tile_pool(name="wbpool", bufs=3) as wbpool,
            tc.tile_pool(name="wdpool", bufs=1) as wdpool,
            tc.tile_pool(name="spool", bufs=2) as spool,
            tc.tile_pool(name="ypool", bufs=3) as ypool,
        ):
            from concourse.tile_rust import add_dep_helper

            # --- front: packed by deadline across the two HWDGE queues ---
            xq_t = [xpool.tile([P, 2 * C], dt.bfloat16, name=f"xq{q}",
                               tag=f"xq{q}") for q in range(8)]

            def load_oct(q, eng):
                return eng.dma_start(out=xq_t[q][:],
                                     in_=xg[:, q * 2 * C:(q + 1) * 2 * C])

            wA0 = wapool.tile([P, 2 * HH, P], dt.bfloat16, name="wA0", tag="wA")
            wB0 = wbpool.tile([P, 2 * HH, P], dt.bfloat16, name="wB0", tag="wB")
            # HWDGE queues accept only 4 outstanding DMA instructions; the
            # 5th trigger blocks the ISSUING ENGINE until the oldest lands.
            # Scalar has compute duties (silu), so it gets exactly 4 front
            # loads; Sync has none, so its 5th+ triggers can block freely.
            with tc.high_priority():
                nc.sync.dma_start(out=wA0[:, :HH, :],                  # wgA it0
                                  in_=wgu[0, :, 0 * HB:1 * HB])
                nc.scalar.dma_start(out=wA0[:, HH:, :],                # wuA it0
                                    in_=wgu[0, :, 1 * HB:2 * HB])
                load_oct(0, nc.sync)                                   # h0-1
                load_oct(1, nc.scalar)                                 # h2-3
                load_oct(2, nc.sync)                                   # h4-5
                nc.gpsimd.dma_start(out=wB0[:],                        # wgB+wuB
                                    in_=wgu[0, :, 2 * HB:])            # it0
                load_oct(3, nc.scalar)                                 # h6-7
                load_oct(4, nc.sync)                                   # h8-9
                load_oct(5, nc.scalar)                                 # h10-11
                load_oct(6, nc.sync)                                   # h12-13
                load_oct(7, nc.scalar)                                 # h14-15
            xg_t = [xq_t[h // 2][:, (h % 2) * C:(h % 2 + 1) * C]
                    for h in range(HT)]

            # PE warm-up while token DMAs stream: tiny matmuls on a zeroed
            # scratch tile release the HAM clock gate (1.2 -> 2.4 GHz takes
            # ~3.4us of sustained PE activity) before real work lands ~11.5us.
            warm = spool.tile([P, 64], dt.bfloat16, name="warm", tag="warm")
            nc.vector.memset(warm[:], 0.0)

            # One PSUM pool spans both phases (phase 2 reuses the pg tags'
            # ring slots) so there is no pool-close barrier at the boundary.
            a_t = []
            with tc.tile_pool(name="pp", bufs=2, space="PSUM") as pp:
                # Phase 1: A[i, t] = silu(G) * U, per 128-row i-tile.
                # Weight loads are emitted one iteration AHEAD so their
                # triggers sit before iteration it's epilogue in the Scalar
                # program (a load emitted at the top of its own iteration
                # would queue behind silu(it-1) and land ~1us late each it).
                wAs, wBs = [wA0], [wB0]
                wd_t = []
                mul_ins = None
                for it in range(IT):
                    if it == 2:
                        # w_down prefetch rides the Sync queue, anchored
                        # behind it=1's epilogue (sync=True) so it cannot
                        # steal aggregate HBM bandwidth (~300 GB/s total)
                        # from the front-critical token/weight deliveries.
                        prev, psync = mul_ins, True
                        for pair in range(HT // 2):
                            w_t = wdpool.tile([P, 2 * IT, P], dt.bfloat16,
                                              name=f"wd{pair}", tag=f"wd{pair}")
                            ins = nc.sync.dma_start(out=w_t[:],
                                                    in_=wdp[pair, :, :])
                            add_dep_helper(ins.ins, prev.ins, sync=psync)
                            prev, psync = ins, False
                            wd_t.append(w_t)
                    if it + 1 < IT:
                        wA = wapool.tile([P, 2 * HH, P], dt.bfloat16,
                                         name=f"wA{it + 1}", tag="wA")
                        nc.gpsimd.dma_start(out=wA[:],
                                            in_=wgu[it + 1, :, :2 * HB])
                        wB = wbpool.tile([P, 2 * HH, P], dt.bfloat16,
                                         name=f"wB{it + 1}", tag="wB")
                        nc.scalar.dma_start(out=wB[:],
                                            in_=wgu[it + 1, :, 2 * HB:])
                        wAs.append(wA)
                        wBs.append(wB)
                    wA, wB = wAs[it], wBs[it]
                    pgs = [pp.tile([P, w], dt.float32, name=f"pg{ci}",
                                   tag=f"pg{ci}", bufs=2)
                           for ci, (s, w) in enumerate(ch)]
                    pus = [pp.tile([P, w], dt.float32, name=f"pu{ci}",
                                   tag=f"pu{ci}", bufs=2)
                           for ci, (s, w) in enumerate(ch)]
                    if it == 0:
                        # bridge PE activity from preamble end (~7.5us) to
                        # first data (~11.5us) so HAM boosts the clock early
                        for _ in range(80):
                            nc.tensor.matmul(pgs[0][:64, :64], warm[:, :],
                                             warm[:, :64], start=True, stop=True)
                    for h in range(HT):
                        st, sp = h == 0, h == HT - 1
                        wh = wA if h < HH else wB
                        wgh = wh[:, h % HH, :]
                        wuh = wh[:, HH + h % HH, :]
                        for ci, (s, w) in enumerate(ch):
                            nc.tensor.matmul(pgs[ci][:], wgh,
                                             xg_t[h][:, s:s + w], start=st, stop=sp)
                        for ci, (s, w) in enumerate(ch):
                            nc.tensor.matmul(pus[ci][:], wuh,
                                             xg_t[h][:, s:s + w], start=st, stop=sp)
                    sg = spool.tile([P, C], dt.float32, name="sg", tag="sg")
                    ai = apool.tile([P, C], dt.bfloat16, name=f"a{it}", tag=f"a{it}")
                    for ci, (s, w) in enumerate(ch):
                        nc.scalar.activation(sg[:, s:s + w], pgs[ci][:],
                                             mybir.ActivationFunctionType.Silu)
                        mul_ins = nc.vector.tensor_mul(ai[:, s:s + w],
                                                       sg[:, s:s + w], pus[ci][:])
                    a_t.append(ai)

                # Phase 2: Y^T[h, t] = sum_i Wd[i, h] * A[i, t]; raw bf16 out.
                # Last h-tile runs chunk-major so chunk 0's copy + store
                # overlap chunk 1's matmuls (shorter drain tail).
                for ht in range(HT):
                    wdt = wd_t[ht // 2]
                    joff = (ht % 2) * IT
                    pys = [pp.tile([P, w], dt.float32, name=f"py{ci}",
                                   tag=f"pg{ci}") for ci, (s, w) in enumerate(ch)]
                    yo = ypool.tile([P, C], dt.bfloat16, name="yo", tag="yo")
                    if ht == HT - 1 and len(ch) > 1:
                        for ci, (s, w) in enumerate(ch):
                            for i2 in range(IT):
                                nc.tensor.matmul(pys[ci][:], wdt[:, joff + i2, :],
                                                 a_t[i2][:, s:s + w],
                                                 start=i2 == 0, stop=i2 == IT - 1)
                            nc.vector.tensor_copy(yo[:, s:s + w], pys[ci][:])
                            nc.scalar.dma_start(
                                out=yt[ht * P:(ht + 1) * P, s:s + w],
                                in_=yo[:, s:s + w])
                    else:
                        for i2 in range(IT):
                            st, sp = i2 == 0, i2 == IT - 1
                            for ci, (s, w) in enumerate(ch):
                                nc.tensor.matmul(pys[ci][:], wdt[:, joff + i2, :],
                                                 a_t[i2][:, s:s + w], start=st, stop=sp)
                        for ci, (s, w) in enumerate(ch):
                            nc.vector.tensor_copy(yo[:, s:s + w], pys[ci][:])
                        nc.scalar.dma_start(out=yt[ht * P:(ht + 1) * P, :], in_=yo[:])
    nc.compile()
    return nc


def _tile_weight(w, nt_out):
    """[K, N] -> [N/128, 128, K] blocks: out[t, p, k*128+c] = w[k*128+p, t*128+c]."""
    K, N = w.shape
    kt = K // P
    return np.ascontiguousarray(
        w.reshape(kt, P, nt_out, P).transpose(2, 1, 0, 3).reshape(nt_out, P, kt * P)
    )


def kernel(hidden_states, gate_w, w_gate, w_up, w_down, top_k):
    global last_results
    hs = np.ascontiguousarray(np.asarray(hidden_states, dtype=np.float32))
    gw = np.asarray(gate_w, dtype=np.float32)
    wg_all = np.asarray(w_gate, dtype=np.float32)
    wu_all = np.asarray(w_up, dtype=np.float32)
    wd_all = np.asarray(w_down, dtype=np.float32)
    K = int(np.asarray(top_k))
    T = hs.shape[0]
    if K <= 0:
        return np.zeros((T, H), np.float32)

    # ---- router (mirrors the reference numerics in fp32) ----
    logits = hs @ gw.T
    m = logits.max(-1, keepdims=True)
    ex = np.exp(logits - m)
    probs = ex / ex.sum(-1, keepdims=True)
    order = np.argsort(-probs, axis=-1, kind="stable")
    topi = order[:, :K]
    topv = np.take_along_axis(probs, topi, axis=-1)
    topv = topv / topv.sum(-1, keepdims=True)

    # ---- dispatch: gather each expert's tokens ----
    idxs, wvs = [], []
    for e in range(E):
        mask = topi == e
        rows = np.nonzero(mask.any(-1))[0]
        idxs.append(rows)
        wvs.append(topv[mask].astype(np.float32))
    counts = [len(r) for r in idxs]
    C = max(64, ((max(counts) + 1) // 2) * 2)

    nc = _compiled.get(C)
    if nc is None:
        nc = _compiled[C] = _build(C)

    bf16 = ml_dtypes.bfloat16
    in_maps = []
    for e in range(E):
        idx = idxs[e]
        n = len(idx)
        xsel = hs[idx]  # [n, H]
        xg_np = np.zeros((HT, P, C), dtype=bf16)
        xg_np[:, :, :n] = xsel.T.astype(bf16).reshape(HT, P, n)
        xg_np = np.ascontiguousarray(xg_np.transpose(1, 0, 2).reshape(P, HT * C))
        wg_t = _tile_weight(wg_all[e].astype(bf16), IT)   # [IT, P, HT*P]
        wu_t = _tile_weight(wu_all[e].astype(bf16), IT)
        A = HH * P
        wgu_np = np.ascontiguousarray(np.concatenate(
            [wg_t[:, :, :A], wu_t[:, :, :A], wg_t[:, :, A:], wu_t[:, :, A:]],
            axis=2))  # [IT, P, 4*A]
        wd_t = _tile_weight(wd_all[e].astype(bf16), HT)   # [HT, P, IT*P]
        wdp_np = np.ascontiguousarray(
            wd_t.reshape(HT // 2, 2, P, IT * P).transpose(0, 2, 1, 3)
            .reshape(HT // 2, P, 2 * IT * P))
        in_maps.append({"xg": xg_np, "wgu": wgu_np, "wdp": wdp_np})

    from concourse.bass_utils import run_bass_kernel_spmd
    res = run_bass_kernel_spmd(nc, in_maps, core_ids=list(range(E)))
    last_results = res

    # ---- combine: apply top-k weights + scatter-add per-expert outputs ----
    out = np.zeros((T, H), np.float32)
    for e in range(E):
        idx = idxs[e]
        n = len(idx)
        yt_e = np.asarray(last_results.results[e]["yt"])  # [H, C] bf16
        out[idx] += wvs[e][:, None] * yt_e[:, :n].T.astype(np.float32)
    return out
